# revision 42
# baseline (speedup 1.0000x reference)
"""Diagonal-matrix multiply (column scale) on 8 Trainium2 NeuronCores.

Computes y = x * weight[None, :]  for x:[8192,4096] f32, weight:[4096] f32.
Data-parallel: rows of x sharded 8 ways (1024 rows/core); weight replicated.

Per-core Bass/Tile kernel: stream eight [128, 4096] f32 tiles (2 MiB each)
HBM->SBUF on the SP HWDGE ring, multiply in place on the vector engine by a
partition-broadcast copy of weight, and stream back on the Activation HWDGE
ring so loads and stores overlap. Memory-bound: 33.5 MB of HBM traffic per
core per pass.
"""

import numpy as np

import concourse.bacc as bacc
import concourse.mybir as mybir
from concourse.tile import TileContext
from concourse.bass_utils import run_bass_kernel_spmd

N_CORES = 8
ROWS = 8192
N = 4096
SHARD_ROWS = ROWS // N_CORES  # 1024 rows per core
P = 128                       # SBUF partitions
N_TILES = SHARD_ROWS // P     # 8 tiles of [128, 4096] (2 MiB) per core

_nc_cache = {}


def _build(repeat=1):
    """Build and Bacc-compile the per-core kernel.

    repeat > 1 wraps the streaming body in a Tile For_i loop that re-runs
    it `repeat` times (idempotent; used only for wall-clock timing).
    NOTE: nc.compile() must run AFTER TileContext exits (scheduling happens
    on exit), and is required — it splits multi-sem waits to satisfy the
    TRN2 one-sync-wait-per-instruction limit.
    """
    if repeat in _nc_cache:
        return _nc_cache[repeat]
    nc = bacc.Bacc()
    x = nc.dram_tensor("x", [SHARD_ROWS, N], mybir.dt.float32, kind="ExternalInput")
    w = nc.dram_tensor("weight", [N], mybir.dt.float32, kind="ExternalInput")
    y = nc.dram_tensor("y", [SHARD_ROWS, N], mybir.dt.float32, kind="ExternalOutput")

    # view [128, 8, 4096]: partition p of tile i holds row i*128 + p
    xv = x.rearrange("(n p) m -> p n m", p=P)
    yv = y.rearrange("(n p) m -> p n m", p=P)

    with TileContext(nc) as tc:
        with (
            tc.tile_pool(name="const", bufs=1) as cpool,
            tc.tile_pool(name="work", bufs=N_TILES) as pool,
        ):
            wtile = cpool.tile([P, N], mybir.dt.float32)
            scratch = cpool.tile([P, 1], mybir.dt.float32)
            # Replicate weight into every partition via a step-0 partition
            # AP on the SWDGE path (keeps both HWDGE rings for the x/y
            # streams).
            nc.gpsimd.dma_start(out=wtile[:, :], in_=w[None, :].to_broadcast([P, N]))
            # Tiny DVE read of wtile: advances DVE's observed tick for the
            # weight DMA sem so the muls below carry a single sync-wait
            # (their own load) instead of two.
            nc.vector.tensor_copy(out=scratch[:, :], in_=wtile[:, :1])
            wb = wtile[:, None, :].to_broadcast([P, 1, N])

            def body():
                for i in range(N_TILES):
                    t = pool.tile([P, 1, N], mybir.dt.float32)
                    nc.sync.dma_start(out=t[:, :, :], in_=xv[:, i:i + 1, :])
                    nc.vector.tensor_mul(out=t[:, :, :], in0=t[:, :, :], in1=wb)
                    nc.scalar.dma_start(out=yv[:, i:i + 1, :], in_=t[:, :, :])

            if repeat == 1:
                body()
            else:
                with tc.For_i(0, repeat, 1):
                    body()
    nc.compile()
    _nc_cache[repeat] = nc
    return nc


def _build_raw(repeat=1):
    """Hand-semaphored variant: identical pipeline, no TileContext, so the
    kernel tail skips Tile's drain + EVSEM butterfly (~10 us fixed cost).

    Loads all issue immediately on the SP HWDGE ring; muls chase loads on
    DVE; stores chase muls on the Activation HWDGE ring. Each instruction
    carries at most one attached wait after Bacc's event-sem splitting.
    `repeat` is ignored (raw loop would need register-based sem targets);
    present only for interface parity.
    """
    key = ("raw", repeat)
    if key in _nc_cache:
        return _nc_cache[key]
    nc = bacc.Bacc()
    x = nc.dram_tensor("x", [SHARD_ROWS, N], mybir.dt.float32, kind="ExternalInput")
    w = nc.dram_tensor("weight", [N], mybir.dt.float32, kind="ExternalInput")
    y = nc.dram_tensor("y", [SHARD_ROWS, N], mybir.dt.float32, kind="ExternalOutput")
    xv = x.rearrange("(n p) m -> p n m", p=P)
    yv = y.rearrange("(n p) m -> p n m", p=P)

    from contextlib import ExitStack

    with ExitStack() as ctx:
        tiles = ctx.enter_context(
            nc.sbuf_tensor("tiles", [P, N_TILES, N], mybir.dt.float32)
        )
        wtile = ctx.enter_context(nc.sbuf_tensor("wtile", [P, N], mybir.dt.float32))
        # one completion sem per load DMA: separate dma_start completions on
        # a shared sem are unordered, so a shared counter would race
        ld_sems = [
            ctx.enter_context(nc.semaphore(f"ld{i}")) for i in range(N_TILES)
        ]
        w_sem = ctx.enter_context(nc.semaphore("w_sem"))
        mul_sem = ctx.enter_context(nc.semaphore("mul_sem"))
        st_sem = ctx.enter_context(nc.semaphore("st_sem"))
        block = ctx.enter_context(nc.Block())

        @block.gpsimd
        def _(gp):
            gp.dma_start(
                out=wtile[:, :], in_=w[None, :].to_broadcast([P, N])
            ).then_inc(w_sem, 16)

        @block.sync
        def _(sync):
            for i in range(N_TILES):
                sync.dma_start(
                    out=tiles[:, i, :], in_=xv[:, i, :]
                ).then_inc(ld_sems[i], 16)

        @block.vector
        def _(vec):
            vec.wait_ge(w_sem, 16)
            for i in range(N_TILES):
                vec.wait_ge(ld_sems[i], 16)
                nc.vector.tensor_mul(
                    out=tiles[:, i, :], in0=tiles[:, i, :], in1=wtile[:, :]
                ).then_inc(mul_sem, 1)

        @block.scalar
        def _(sc):
            for i in range(N_TILES):
                sc.wait_ge(mul_sem, i + 1)
                sc.dma_start(
                    out=yv[:, i, :], in_=tiles[:, i, :]
                ).then_inc(st_sem, 16)
            # all store completions is a pure barrier, so one shared sem is
            # fine here; ensures every byte of y landed before kernel exit
            sc.wait_ge(st_sem, 16 * N_TILES)

    nc.compile()
    _nc_cache[key] = nc
    return nc


def _shard_inputs(x, weight):
    x = np.ascontiguousarray(np.asarray(x, dtype=np.float32))
    weight = np.ascontiguousarray(np.asarray(weight, dtype=np.float32))
    shards = np.split(x, N_CORES, axis=0)
    return [{"x": s, "weight": weight} for s in shards]


def _run(x, weight, repeat=1, **spmd_kwargs):
    # graded single-shot path uses the raw build (no Tile tail overhead);
    # repeat>1 timing builds need Tile's For_i, so they use _build()
    nc = _build_raw() if repeat == 1 else _build(repeat)
    in_maps = _shard_inputs(x, weight)
    res = run_bass_kernel_spmd(nc, in_maps, list(range(N_CORES)), **spmd_kwargs)
    out = np.concatenate([np.asarray(r["y"]) for r in res.results], axis=0)
    return out.astype(np.float32, copy=False), res


def kernel(x, weight):
    out, _ = _run(x, weight)
    return out


# revision 43
# speedup vs baseline: 1.0990x; 1.0990x over previous
"""Diagonal-matrix multiply (column scale) on 8 Trainium2 NeuronCores.

Computes y = x * weight[None, :]  for x:[8192,4096] f32, weight:[4096] f32.
Data-parallel: rows of x sharded 8 ways (1024 rows/core); weight replicated.

Per-core Bass/Tile kernel: stream eight [128, 4096] f32 tiles (2 MiB each)
HBM->SBUF on the SP HWDGE ring, multiply in place on the vector engine by a
partition-broadcast copy of weight, and stream back on the Activation HWDGE
ring so loads and stores overlap. Memory-bound: 33.5 MB of HBM traffic per
core per pass.
"""

import numpy as np

import concourse.bacc as bacc
import concourse.mybir as mybir
from concourse.tile import TileContext
from concourse.bass_utils import run_bass_kernel_spmd

N_CORES = 8
ROWS = 8192
N = 4096
SHARD_ROWS = ROWS // N_CORES  # 1024 rows per core
P = 128                       # SBUF partitions
N_TILES = SHARD_ROWS // P     # 8 tiles of [128, 4096] (2 MiB) per core

_nc_cache = {}


def _build(repeat=1):
    """Build and Bacc-compile the per-core kernel.

    repeat > 1 wraps the streaming body in a Tile For_i loop that re-runs
    it `repeat` times (idempotent; used only for wall-clock timing).
    NOTE: nc.compile() must run AFTER TileContext exits (scheduling happens
    on exit), and is required — it splits multi-sem waits to satisfy the
    TRN2 one-sync-wait-per-instruction limit.
    """
    if repeat in _nc_cache:
        return _nc_cache[repeat]
    nc = bacc.Bacc()
    x = nc.dram_tensor("x", [SHARD_ROWS, N], mybir.dt.float32, kind="ExternalInput")
    w = nc.dram_tensor("weight", [N], mybir.dt.float32, kind="ExternalInput")
    y = nc.dram_tensor("y", [SHARD_ROWS, N], mybir.dt.float32, kind="ExternalOutput")

    # view [128, 8, 4096]: partition p of tile i holds row i*128 + p
    xv = x.rearrange("(n p) m -> p n m", p=P)
    yv = y.rearrange("(n p) m -> p n m", p=P)

    with TileContext(nc) as tc:
        with (
            tc.tile_pool(name="const", bufs=1) as cpool,
            tc.tile_pool(name="work", bufs=N_TILES) as pool,
        ):
            wtile = cpool.tile([P, N], mybir.dt.float32)
            scratch = cpool.tile([P, 1], mybir.dt.float32)
            # Replicate weight into every partition via a step-0 partition
            # AP on the SWDGE path (keeps both HWDGE rings for the x/y
            # streams).
            nc.gpsimd.dma_start(out=wtile[:, :], in_=w[None, :].to_broadcast([P, N]))
            # Tiny DVE read of wtile: advances DVE's observed tick for the
            # weight DMA sem so the muls below carry a single sync-wait
            # (their own load) instead of two.
            nc.vector.tensor_copy(out=scratch[:, :], in_=wtile[:, :1])
            wb = wtile[:, None, :].to_broadcast([P, 1, N])

            def body():
                for i in range(N_TILES):
                    t = pool.tile([P, 1, N], mybir.dt.float32)
                    nc.sync.dma_start(out=t[:, :, :], in_=xv[:, i:i + 1, :])
                    nc.vector.tensor_mul(out=t[:, :, :], in0=t[:, :, :], in1=wb)
                    nc.scalar.dma_start(out=yv[:, i:i + 1, :], in_=t[:, :, :])

            if repeat == 1:
                body()
            else:
                with tc.For_i(0, repeat, 1):
                    body()
    nc.compile()
    _nc_cache[repeat] = nc
    return nc


def _build_sync(repeat=1, sync=True):
    """Experimental: phase-separated kernel with a chip-wide AllReduce
    barrier at the phase boundary. Aligns all 8 cores so every HBM domain
    sees pure reads, then pure writes (cores drifting out of phase re-mix
    directions at the domain level even when each core is phase-separated).
    sync=False gives the same structure without the collective (control).
    """
    key = ("sync", repeat, sync)
    if key in _nc_cache:
        return _nc_cache[key]
    nc = bacc.Bacc()
    x = nc.dram_tensor("x", [SHARD_ROWS, N], mybir.dt.float32, kind="ExternalInput")
    w = nc.dram_tensor("weight", [N], mybir.dt.float32, kind="ExternalInput")
    y = nc.dram_tensor("y", [SHARD_ROWS, N], mybir.dt.float32, kind="ExternalOutput")
    xv = x.rearrange("(n p) m -> p n m", p=P)
    yv = y.rearrange("(n p) m -> p n m", p=P)
    cc_in = nc.dram_tensor("cc_in", [1, 1], mybir.dt.float32)
    cc_out = nc.dram_tensor("cc_out", [1, 1], mybir.dt.float32)

    with TileContext(nc) as tc:
        with (
            tc.tile_pool(name="const", bufs=1) as cpool,
            tc.tile_pool(name="work", bufs=N_TILES) as pool,
        ):
            wtile = cpool.tile([P, N], mybir.dt.float32)
            scratch = cpool.tile([P, 1], mybir.dt.float32)
            nc.gpsimd.dma_start(out=wtile[:, :], in_=w[None, :].to_broadcast([P, N]))
            nc.vector.tensor_copy(out=scratch[:, :], in_=wtile[:, :1])
            wb = wtile[:, None, :].to_broadcast([P, 1, N])

            def body():
                tiles = []
                for i in range(N_TILES):
                    t = pool.tile([P, 1, N], mybir.dt.float32)
                    nc.sync.dma_start(out=t[:, :, :], in_=xv[:, i:i + 1, :])
                    nc.vector.tensor_mul(out=t[:, :, :], in0=t[:, :, :], in1=wb)
                    tiles.append(t)
                tc.strict_bb_all_engine_barrier()
                if sync:
                    nc.gpsimd.collective_compute(
                        "AllReduce",
                        mybir.AluOpType.add,
                        replica_groups=[list(range(N_CORES))],
                        ins=[cc_in[:]],
                        outs=[cc_out[:]],
                    )
                    tc.strict_bb_all_engine_barrier()
                for i, t in enumerate(tiles):
                    nc.scalar.dma_start(out=yv[:, i:i + 1, :], in_=t[:, :, :])
                tc.strict_bb_all_engine_barrier()

            if repeat == 1:
                body()
            else:
                with tc.For_i(0, repeat, 1):
                    body()
    nc.compile()
    _nc_cache[key] = nc
    return nc


def _build_raw(repeat=1):
    """Hand-semaphored variant: identical pipeline, no TileContext, so the
    kernel tail skips Tile's drain + EVSEM butterfly (~10 us fixed cost).

    Loads all issue immediately on the SP HWDGE ring; muls chase loads on
    DVE; stores chase muls on the Activation HWDGE ring. Each instruction
    carries at most one attached wait after Bacc's event-sem splitting.
    `repeat` is ignored (raw loop would need register-based sem targets);
    present only for interface parity.
    """
    key = ("raw", repeat)
    if key in _nc_cache:
        return _nc_cache[key]
    nc = bacc.Bacc()
    x = nc.dram_tensor("x", [SHARD_ROWS, N], mybir.dt.float32, kind="ExternalInput")
    w = nc.dram_tensor("weight", [N], mybir.dt.float32, kind="ExternalInput")
    y = nc.dram_tensor("y", [SHARD_ROWS, N], mybir.dt.float32, kind="ExternalOutput")
    xv = x.rearrange("(n p) m -> p n m", p=P)
    yv = y.rearrange("(n p) m -> p n m", p=P)

    from contextlib import ExitStack

    with ExitStack() as ctx:
        tiles = ctx.enter_context(
            nc.sbuf_tensor("tiles", [P, N_TILES, N], mybir.dt.float32)
        )
        wtile = ctx.enter_context(nc.sbuf_tensor("wtile", [P, N], mybir.dt.float32))
        # one completion sem per load DMA: separate dma_start completions on
        # a shared sem are unordered, so a shared counter would race
        ld_sems = [
            ctx.enter_context(nc.semaphore(f"ld{i}")) for i in range(N_TILES)
        ]
        w_sem = ctx.enter_context(nc.semaphore("w_sem"))
        mul_sem = ctx.enter_context(nc.semaphore("mul_sem"))
        st_sem = ctx.enter_context(nc.semaphore("st_sem"))
        block = ctx.enter_context(nc.Block())

        @block.gpsimd
        def _(gp):
            gp.dma_start(
                out=wtile[:, :], in_=w[None, :].to_broadcast([P, N])
            ).then_inc(w_sem, 16)

        @block.sync
        def _(sync):
            for i in range(N_TILES):
                sync.dma_start(
                    out=tiles[:, i, :], in_=xv[:, i, :]
                ).then_inc(ld_sems[i], 16)

        @block.vector
        def _(vec):
            vec.wait_ge(w_sem, 16)
            for i in range(N_TILES):
                vec.wait_ge(ld_sems[i], 16)
                nc.vector.tensor_mul(
                    out=tiles[:, i, :], in0=tiles[:, i, :], in1=wtile[:, :]
                ).then_inc(mul_sem, 1)

        @block.scalar
        def _(sc):
            for i in range(N_TILES):
                sc.wait_ge(mul_sem, i + 1)
                sc.dma_start(
                    out=yv[:, i, :], in_=tiles[:, i, :]
                ).then_inc(st_sem, 16)
            # all store completions is a pure barrier, so one shared sem is
            # fine here; ensures every byte of y landed before kernel exit
            sc.wait_ge(st_sem, 16 * N_TILES)

    nc.compile()
    _nc_cache[key] = nc
    return nc


def _shard_inputs(x, weight):
    x = np.ascontiguousarray(np.asarray(x, dtype=np.float32))
    weight = np.ascontiguousarray(np.asarray(weight, dtype=np.float32))
    shards = np.split(x, N_CORES, axis=0)
    return [{"x": s, "weight": weight} for s in shards]


def _run(x, weight, repeat=1, **spmd_kwargs):
    # graded single-shot path uses the raw build (no Tile tail overhead);
    # repeat>1 timing builds need Tile's For_i, so they use _build()
    nc = _build_raw() if repeat == 1 else _build(repeat)
    in_maps = _shard_inputs(x, weight)
    res = run_bass_kernel_spmd(nc, in_maps, list(range(N_CORES)), **spmd_kwargs)
    out = np.concatenate([np.asarray(r["y"]) for r in res.results], axis=0)
    return out.astype(np.float32, copy=False), res


def kernel(x, weight):
    out, _ = _run(x, weight)
    return out
